# revision 27
# baseline (speedup 1.0000x reference)
"""BinaryDilGroupConv Trainium2 kernel (v4).

Computes, for x[N=64, C=256, 32, 32]:
    h = BN(x)  (inference affine)
    a = sign(h); w = sign(weight)
    y = grouped dilated conv(a, w; groups=64, k=3, dil=2, pad=2)
    out = channel_shuffle(y, g=64) + x

Sharding: data-parallel over batch N across 8 NeuronCores (8 samples/core).
Params replicated. No collectives.

v4 design:
  - Activations live at row pitch 32 (no x padding), as THREE copies, one
    per dx tap column-shift (built from the Sign output with two flat
    1150-byte shifted copies + small edge memsets). Flat matmul windows
    are then EXACT: 512 columns per 16-row chunk, zero junk, and every
    window offset is a multiple of 32 so any two taps can form an fp8
    DoubleRow pair (step%16==0). 9 taps run as 4 DoubleRow pairs + 1
    single = 5 passes of N=512 (vs 6 passes of N=320+junk in v1-v3).
  - PSUM eviction + channel shuffle + residual add stay fused in one DVE
    tensor_add per sample: psum holds couts at m = 32j+g (shuffled final
    channel 64j+32h+g); a second copy of x is uploaded host-permuted to
    that order, so the add lines up partition-for-partition, and the
    host un-permutes the bf16 output after download.
  - All per-pair DRAM I/O is ONE dma trigger (host pre-permutes layouts;
    queue-side trigger cost ~0.7us dominates descriptor shape).
  - x for the sign path is fp16 (sign flips vs fp32: ~1e-5 of elements,
    rel err ~5e-3 << 2e-2); residual copy + output are bf16.
"""

import numpy as np
import ml_dtypes

C = 256
G = 64            # groups
CPG = 4           # channels per group
K = 3
DIL = 2
PAD = 2
EPS = 1e-5
H = W = 32
S = H * W         # 1024 spatial positions
PITCH = 32        # activation row pitch (exact, no x padding)
CPH = 36          # copy rows: 2 pad + 32 + 2 pad
CSZ = CPH * PITCH  # 1152 bytes per dx-copy
ACT_OFF = CSZ + 2 * PITCH   # interior of copy 1 (the unshifted one)
N_FULL = 64
N_CORES = 8
NS = N_FULL // N_CORES   # samples per core
NHALF = 2                # channel halves of 128
NCHUNK = 2               # psum chunks per half (16 output rows each)
CROWS = H // NCHUNK      # 16 rows per chunk
NMM = CROWS * PITCH      # 512 columns per matmul (one psum bank)
ABUFS = 3                # activation-copy round-robin depth
NPAIR = NS // 2

# tap slot order: 4 DoubleRow pairs + 1 single; (dy, dx) per slot
SLOT_MAP = [(0, 0), (1, 0), (0, 1), (1, 1), (0, 2), (1, 2),
            (2, 0), (2, 1), (2, 2)]

_COMPILED = None


def build(n_samples=NS):
    """Build + compile the per-core Bass program."""
    import concourse.bass as bass
    import concourse.bacc as bacc
    import concourse.tile as tile
    import concourse.mybir as mybir

    fp32 = mybir.dt.float32
    bf16 = mybir.dt.bfloat16
    fp16 = mybir.dt.float16
    fp8 = mybir.dt.float8e4

    nc = bacc.Bacc("TRN2", target_bir_lowering=False, debug=False,
                   num_devices=N_CORES)

    # sign path: natural channel order, partition-major [p=ch%128, h, s]
    xin = nc.dram_tensor("xin", [n_samples, 128, NHALF, S], fp16,
                         kind="ExternalInput").ap()
    # residual path: host-permuted to psum order (partition m = 32j+g,
    # slot h holds channel 64j+32h+g)
    xres = nc.dram_tensor("xres", [n_samples, 128, NHALF, S], bf16,
                          kind="ExternalInput").ap()
    # weight free index = h*9 + slot (see SLOT_MAP)
    wT = nc.dram_tensor("wT", [128, NHALF * K * K, 128], fp8,
                        kind="ExternalInput").ap()
    bnsc = nc.dram_tensor("bnsc", [NHALF, 128], fp32,
                          kind="ExternalInput").ap()
    bnbi = nc.dram_tensor("bnbi", [NHALF, 128], fp32,
                          kind="ExternalInput").ap()
    # output in psum order; host un-permutes
    out = nc.dram_tensor("out", [n_samples, 128, NHALF, S], bf16,
                         kind="ExternalOutput").ap()

    with tile.TileContext(nc) as tc:
        with (
            tc.tile_pool(name="const", bufs=1) as constp,
            tc.tile_pool(name="xnp", bufs=NPAIR) as xnp,
            tc.tile_pool(name="xrp", bufs=NPAIR) as xrp,
            tc.tile_pool(name="finp", bufs=2) as finp,
            tc.tile_pool(name="psum", bufs=2, space="PSUM") as psump,
        ):
            xn_t = {}
            xr_t = {}

            def load_xn(pi, split=False):
                xn_t[pi] = xnp.tile([128, 2, NHALF, S], fp16, name="xn",
                                    tag="xn")
                src = xin[2 * pi:2 * pi + 2].rearrange("n p h s -> p n h s")
                if split:
                    # sample 0 h0 races ahead, split across two rings;
                    # the rest of the pair follows on the third
                    nc.sync.dma_start(xn_t[pi][:, 0, 0, 0:S // 2],
                                      src[:, 0, 0, 0:S // 2])
                    nc.gpsimd.dma_start(xn_t[pi][:, 0, 0, S // 2:S],
                                        src[:, 0, 0, S // 2:S])
                    nc.sync.dma_start(xn_t[pi][:, 0, 1, :],
                                      src[:, 0, 1, :])
                    nc.scalar.dma_start(xn_t[pi][:, 1, :, :],
                                        src[:, 1, :, :])
                else:
                    nc.scalar.dma_start(xn_t[pi][:], src)

            def load_xr(pi):
                xr_t[pi] = xrp.tile([128, 2, NHALF, S], bf16, name="xr",
                                    tag="xr")
                # split across sync/scalar rings: gpsimd must stay clear
                # for the latency-critical shift DMAs
                src = xres[2 * pi:2 * pi + 2].rearrange("n p h s -> p n h s")
                nc.sync.dma_start(xr_t[pi][:, 0, :, :], src[:, 0, :, :])
                nc.scalar.dma_start(xr_t[pi][:, 1, :, :], src[:, 1, :, :])

            sc_tile = constp.tile([128, NHALF], fp32)
            nc.scalar.dma_start(sc_tile[:], bnsc.rearrange("h p -> p h"))
            bi_tile = constp.tile([128, NHALF], fp32)
            nc.scalar.dma_start(bi_tile[:], bnbi.rearrange("h p -> p h"))
            load_xn(0, split=True)
            w_tile = constp.tile([128, NHALF * K * K, 128], fp8)
            nc.scalar.dma_start(w_tile[:], wT)

            # warmup: ACT table load + PE busy until the first real MM so
            # HAM is un-throttled by then
            warm_sb = constp.tile([128, 512], fp8)
            nc.gpsimd.memset(warm_sb[:], 0.0)
            warm_w = constp.tile([128, 128], fp8)
            nc.gpsimd.memset(warm_w[:], 0.0)
            warm_act = constp.tile([128, 16], fp8)
            nc.scalar.activation(warm_act[:], warm_sb[:, 0:16],
                                 mybir.ActivationFunctionType.Sign)
            warm_ps = psump.tile([128, NHALF, NCHUNK, NMM], fp32,
                                 name="ps", tag="ps")
            for i in range(8):
                nc.tensor.matmul(warm_ps[:, i % 2, (i // 2) % 2, :],
                                 warm_w[:], warm_sb[:],
                                 start=True, stop=True)
            for i in range(4):
                nc.tensor.matmul(warm_ps[:, i % 2, (i // 2) % 2, :],
                                 w_tile[:, 0, :], warm_sb[:],
                                 start=True, stop=True)

            # ---- activation copies: [copy0 | copy1 | copy2], pitch 32.
            # copy k serves dx=k taps: copy_k[r, x] = a_img[r-2, x+2k-2].
            # copy1 rows 0,1,34,35 are static zeros; the flat shifted
            # copies propagate them; edge cols are re-zeroed per sample.
            acps = [[constp.tile([128, 3 * CSZ], fp8, name=f"ac{h}_{b}")
                     for b in range(ABUFS)] for h in range(NHALF)]
            for h in range(NHALF):
                for b in range(ABUFS):
                    a = acps[h][b]
                    nc.gpsimd.memset(a[:, CSZ:CSZ + 2 * PITCH], 0.0)
                    nc.gpsimd.memset(a[:, 2 * CSZ - 2 * PITCH:2 * CSZ], 0.0)

            # (pair 0/1 bulk loads are issued inside the loop, after the
            # first Signs, so they don't clog the engine queues ahead of
            # the latency-critical first activations)

            def window(acp, offset, step, ncols):
                """Flat window AP [128, (2,)? ncols] of the copies tile;
                step is the DoubleRow pair stride (None for single)."""
                base = acp[:, offset:offset + 1]
                ap = [list(acp[:].ap[0])]
                if step is not None:
                    ap.append([step, 2])
                ap.append([1, ncols])
                return bass.AP(base.tensor, base.offset, ap)

            for n in range(n_samples):
                pi, sl = divmod(n, 2)
                xn = xn_t[pi]

                ps = psump.tile([128, NHALF, NCHUNK, NMM], fp32,
                                name="ps", tag="ps")
                for h in range(NHALF):
                    a = acps[h][n % ABUFS]
                    a3 = a[:].rearrange("p (k r x) -> p k r x", k=3,
                                        x=PITCH)
                    # a = Sign(x*scale + bias), fp8, into copy1 interior
                    nc.scalar.activation(
                        a[:, ACT_OFF:ACT_OFF + S],
                        xn[:, sl, h, :],
                        mybir.ActivationFunctionType.Sign,
                        bias=bi_tile[:, h:h + 1],
                        scale=sc_tile[:, h:h + 1],
                    )
                    # flat shifted copies (SBUF->SBUF DMA: one contiguous
                    # 1150B run per partition; engine copies are ~3.6
                    # ns/elem, far too slow) + edge re-zeroing
                    nc.gpsimd.dma_start(a[:, 2:CSZ],
                                        a[:, CSZ:2 * CSZ - 2])
                    nc.gpsimd.dma_start(a[:, 2 * CSZ:3 * CSZ - 2],
                                        a[:, CSZ + 2:2 * CSZ])
                    nc.gpsimd.memset(a3[:, 2, :, W - 2:W], 0.0)
                    nc.vector.memset(a3[:, 0, :, 0:2], 0.0)

                # stagger bulk loads now that this sample's activation
                # chain is queued
                if n == 0:
                    load_xr(0)
                    load_xn(1)
                    load_xr(1)
                elif n == 1:
                    load_xn(2)
                    load_xr(2)
                elif sl == 1 and pi + 2 < NPAIR:
                    load_xn(pi + 2)
                    load_xr(pi + 2)

                # conv: 4 DoubleRow pairs + 1 single, pass-major so
                # consecutive MMs share lhsT; N=512 per chunk
                for h in range(NHALF):
                    a = acps[h][n % ABUFS]
                    for p in range(4):
                        dy0, dx0 = SLOT_MAP[2 * p]
                        dy1, dx1 = SLOT_MAP[2 * p + 1]
                        off0 = dx0 * CSZ + 2 * dy0 * PITCH
                        step = (dx1 * CSZ + 2 * dy1 * PITCH) - off0
                        for c in range(NCHUNK):
                            nc.tensor.matmul(
                                ps[:, h, c, :],
                                w_tile[:, h * 9 + 2 * p:h * 9 + 2 * p + 2,
                                       :],
                                window(a, off0 + c * NMM, step, NMM),
                                start=(p == 0), stop=False,
                                perf_mode=mybir.MatmulPerfMode.DoubleRow,
                            )
                    dy0, dx0 = SLOT_MAP[8]
                    off0 = dx0 * CSZ + 2 * dy0 * PITCH
                    for c in range(NCHUNK):
                        nc.tensor.matmul(
                            ps[:, h, c, :],
                            w_tile[:, h * 9 + 8, :],
                            window(a, off0 + c * NMM, None, NMM),
                            start=False, stop=(c == NCHUNK - 1),
                        )

                # fused eviction + shuffle + residual (psum is exact and
                # contiguous: [h, c, 512] matches [h, s] linear order)
                if sl == 0:
                    fin = finp.tile([128, 2, NHALF, S], bf16, name="fin",
                                    tag="fin")
                else:
                    fin = fin_prev
                fin_prev = fin
                xr = xr_t[pi]
                for h in range(NHALF):
                    nc.vector.tensor_add(
                        fin[:, sl, h, :],
                        ps[:, h, :, :].rearrange("p c b -> p (c b)"),
                        xr[:, sl, h, :])
                    # per-half store overlaps the other half's add (tail)
                    nc.sync.dma_start(out[n, :, h, :], fin[:, sl, h, :])
                if sl == 1:
                    xn_t.pop(pi)
                    xr_t.pop(pi)

    nc.compile()
    return nc


def _host_prep(x, weight, gamma, beta, running_mean, running_var):
    """Precompute BN affine + block-diagonal signed weights."""
    inv = (gamma / np.sqrt(running_var + EPS)).astype(np.float32)
    bias = (beta - running_mean * inv).astype(np.float32)
    wsign = np.sign(weight).astype(np.float32)   # [256, 4, 3, 3]

    lhsT = np.zeros((NHALF, K * K, 128, 128), np.float32)
    # Column m of lhsT (-> PSUM partition m) holds cout co = 4*(m%32)+m//32
    # within the half, so PSUM partition order is m = 32j + g for conv
    # cout 4g + j; the shuffled final channel is then 64j + 32h + g.
    m = np.arange(128)
    co = CPG * (m % 32) + m // 32
    gl = co // CPG
    for h in range(NHALF):
        for s, (dy, dx) in enumerate(SLOT_MAP):
            for kk in range(CPG):
                lhsT[h, s, CPG * gl + kk, m] = wsign[128 * h + co, kk,
                                                     dy, dx]
    lhsT = np.ascontiguousarray(
        lhsT.astype(ml_dtypes.float8_e4m3)
        .transpose(2, 0, 1, 3)
        .reshape(128, NHALF * K * K, 128))
    sc = np.ascontiguousarray(inv.reshape(NHALF, 128))
    bi = np.ascontiguousarray(bias.reshape(NHALF, 128))
    return lhsT, sc, bi


def _get_compiled():
    global _COMPILED
    if _COMPILED is None:
        _COMPILED = build(NS)
    return _COMPILED


# channel permutation: psum partition m, slot h <-> channel 64j + 32h + g
_PM = np.arange(128)
_PERM = (64 * (_PM[:, None] // 32) + 32 * np.arange(2)[None, :]
         + _PM[:, None] % 32)          # [128, 2] -> channel index


def make_in_maps(x, weight, gamma, beta, running_mean, running_var):
    lhsT, sc, bi = _host_prep(x, weight, gamma, beta, running_mean,
                              running_var)
    xs = x.astype(np.float32).reshape(N_CORES, NS, C, S)
    # sign path: natural, partition-major [ns, 128, 2, S]
    xns = np.ascontiguousarray(
        xs.astype(np.float16).reshape(N_CORES, NS, NHALF, 128, S)
        .transpose(0, 1, 3, 2, 4))
    # residual path: psum-order permuted [ns, 128, 2, S]
    xrs = np.ascontiguousarray(
        xs.astype(ml_dtypes.bfloat16)[:, :, _PERM, :])
    return [
        {"xin": xns[i], "xres": xrs[i], "wT": lhsT, "bnsc": sc, "bnbi": bi}
        for i in range(N_CORES)
    ]


def kernel(x, weight, gamma, beta, running_mean, running_var):
    from concourse.bass_utils import run_bass_kernel_spmd

    nc = _get_compiled()
    in_maps = make_in_maps(np.asarray(x), np.asarray(weight),
                           np.asarray(gamma), np.asarray(beta),
                           np.asarray(running_mean), np.asarray(running_var))
    res = run_bass_kernel_spmd(nc, in_maps, list(range(N_CORES)))
    outs = []
    for i in range(N_CORES):
        od = res.results[i]["out"].astype(np.float32)   # [NS, 128, 2, S]
        on = np.empty((NS, C, S), np.float32)
        on[:, _PERM.reshape(-1), :] = od.reshape(NS, 256, S)
        outs.append(on.reshape(NS, C, H, W))
    return np.concatenate(outs, axis=0)


# revision 32
# speedup vs baseline: 1.0075x; 1.0075x over previous
"""BinaryDilGroupConv Trainium2 kernel (v4).

Computes, for x[N=64, C=256, 32, 32]:
    h = BN(x)  (inference affine)
    a = sign(h); w = sign(weight)
    y = grouped dilated conv(a, w; groups=64, k=3, dil=2, pad=2)
    out = channel_shuffle(y, g=64) + x

Sharding: data-parallel over batch N across 8 NeuronCores (8 samples/core).
Params replicated. No collectives.

v4 design:
  - Activations live at row pitch 32 (no x padding), as THREE copies, one
    per dx tap column-shift (built from the Sign output with two flat
    1150-byte shifted copies + small edge memsets). Flat matmul windows
    are then EXACT: 512 columns per 16-row chunk, zero junk, and every
    window offset is a multiple of 32 so any two taps can form an fp8
    DoubleRow pair (step%16==0). 9 taps run as 4 DoubleRow pairs + 1
    single = 5 passes of N=512 (vs 6 passes of N=320+junk in v1-v3).
  - PSUM eviction + channel shuffle + residual add stay fused in one DVE
    tensor_add per sample: psum holds couts at m = 32j+g (shuffled final
    channel 64j+32h+g); a second copy of x is uploaded host-permuted to
    that order, so the add lines up partition-for-partition, and the
    host un-permutes the bf16 output after download.
  - All per-pair DRAM I/O is ONE dma trigger (host pre-permutes layouts;
    queue-side trigger cost ~0.7us dominates descriptor shape).
  - x for the sign path is fp16 (sign flips vs fp32: ~1e-5 of elements,
    rel err ~5e-3 << 2e-2); residual copy + output are bf16.
"""

import numpy as np
import ml_dtypes

C = 256
G = 64            # groups
CPG = 4           # channels per group
K = 3
DIL = 2
PAD = 2
EPS = 1e-5
H = W = 32
S = H * W         # 1024 spatial positions
PITCH = 32        # activation row pitch (exact, no x padding)
CPH = 36          # copy rows: 2 pad + 32 + 2 pad
CSZ = CPH * PITCH  # 1152 bytes per dx-copy
ACT_OFF = CSZ + 2 * PITCH   # interior of copy 1 (the unshifted one)
N_FULL = 64
N_CORES = 8
NS = N_FULL // N_CORES   # samples per core
NHALF = 2                # channel halves of 128
NCHUNK = 2               # psum chunks per half (16 output rows each)
CROWS = H // NCHUNK      # 16 rows per chunk
NMM = CROWS * PITCH      # 512 columns per matmul (one psum bank)
ABUFS = 3                # activation-copy round-robin depth
NPAIR = NS // 2

# tap slot order: 4 DoubleRow pairs + 1 single; (dy, dx) per slot
SLOT_MAP = [(0, 0), (1, 0), (0, 1), (1, 1), (0, 2), (1, 2),
            (2, 0), (2, 1), (2, 2)]

_COMPILED = None


def build(n_samples=NS):
    """Build + compile the per-core Bass program."""
    import concourse.bass as bass
    import concourse.bacc as bacc
    import concourse.tile as tile
    import concourse.mybir as mybir

    fp32 = mybir.dt.float32
    bf16 = mybir.dt.bfloat16
    fp16 = mybir.dt.float16
    fp8 = mybir.dt.float8e4

    nc = bacc.Bacc("TRN2", target_bir_lowering=False, debug=False,
                   num_devices=N_CORES)

    # sign path: natural channel order, partition-major [p=ch%128, h, s]
    xin = nc.dram_tensor("xin", [n_samples, 128, NHALF, S], fp16,
                         kind="ExternalInput").ap()
    # residual path: host-permuted to psum order (partition m = 32j+g,
    # slot h holds channel 64j+32h+g)
    xres = nc.dram_tensor("xres", [n_samples, 128, NHALF, S], bf16,
                          kind="ExternalInput").ap()
    # weight free index = h*9 + slot (see SLOT_MAP)
    wT = nc.dram_tensor("wT", [128, NHALF * K * K, 128], fp8,
                        kind="ExternalInput").ap()
    bnsc = nc.dram_tensor("bnsc", [NHALF, 128], fp32,
                          kind="ExternalInput").ap()
    bnbi = nc.dram_tensor("bnbi", [NHALF, 128], fp32,
                          kind="ExternalInput").ap()
    # output in psum order; host un-permutes
    out = nc.dram_tensor("out", [n_samples, 128, NHALF, S], bf16,
                         kind="ExternalOutput").ap()

    with tile.TileContext(nc) as tc:
        with (
            tc.tile_pool(name="const", bufs=1) as constp,
            tc.tile_pool(name="xnp", bufs=NPAIR) as xnp,
            tc.tile_pool(name="xrp", bufs=NPAIR) as xrp,
            tc.tile_pool(name="finp", bufs=2) as finp,
            tc.tile_pool(name="psum", bufs=2, space="PSUM") as psump,
        ):
            xn_t = {}
            xr_t = {}

            def load_xn(pi, split=False):
                xn_t[pi] = xnp.tile([128, 2, NHALF, S], fp16, name="xn",
                                    tag="xn")
                src = xin[2 * pi:2 * pi + 2].rearrange("n p h s -> p n h s")
                if split:
                    # sample 0 h0 races ahead, split across two rings;
                    # the rest of the pair follows
                    nc.sync.dma_start(xn_t[pi][:, 0, 0, 0:S // 2],
                                      src[:, 0, 0, 0:S // 2])
                    nc.gpsimd.dma_start(xn_t[pi][:, 0, 0, S // 2:S],
                                        src[:, 0, 0, S // 2:S])
                    nc.sync.dma_start(xn_t[pi][:, 0, 1, :],
                                      src[:, 0, 1, :])
                    nc.scalar.dma_start(xn_t[pi][:, 1, :, :],
                                        src[:, 1, :, :])
                else:
                    nc.gpsimd.dma_start(xn_t[pi][:], src)

            def load_xr(pi):
                xr_t[pi] = xrp.tile([128, 2, NHALF, S], bf16, name="xr",
                                    tag="xr")
                nc.gpsimd.dma_start(
                    xr_t[pi][:],
                    xres[2 * pi:2 * pi + 2].rearrange("n p h s -> p n h s"))

            sc_tile = constp.tile([128, NHALF], fp32)
            nc.scalar.dma_start(sc_tile[:], bnsc.rearrange("h p -> p h"))
            bi_tile = constp.tile([128, NHALF], fp32)
            nc.scalar.dma_start(bi_tile[:], bnbi.rearrange("h p -> p h"))
            load_xn(0, split=True)
            w_tile = constp.tile([128, NHALF * K * K, 128], fp8)
            nc.scalar.dma_start(w_tile[:], wT)

            # warmup: ACT table load + PE busy until the first real MM so
            # HAM is un-throttled by then
            warm_sb = constp.tile([128, 512], fp8)
            nc.gpsimd.memset(warm_sb[:], 0.0)
            warm_w = constp.tile([128, 128], fp8)
            nc.gpsimd.memset(warm_w[:], 0.0)
            warm_act = constp.tile([128, 16], fp8)
            nc.scalar.activation(warm_act[:], warm_sb[:, 0:16],
                                 mybir.ActivationFunctionType.Sign)
            warm_ps = psump.tile([128, NHALF, NCHUNK, NMM], fp32,
                                 name="ps", tag="ps")
            for i in range(8):
                nc.tensor.matmul(warm_ps[:, i % 2, (i // 2) % 2, :],
                                 warm_w[:], warm_sb[:],
                                 start=True, stop=True)
            for i in range(4):
                nc.tensor.matmul(warm_ps[:, i % 2, (i // 2) % 2, :],
                                 w_tile[:, 0, :], warm_sb[:],
                                 start=True, stop=True)

            # ---- activation copies: [copy0 | copy1 | copy2], pitch 32.
            # copy k serves dx=k taps: copy_k[r, x] = a_img[r-2, x+2k-2].
            # copy1 rows 0,1,34,35 are static zeros; the flat shifted
            # copies propagate them; edge cols are re-zeroed per sample.
            acps = [[constp.tile([128, 3 * CSZ], fp8, name=f"ac{h}_{b}")
                     for b in range(ABUFS)] for h in range(NHALF)]
            for h in range(NHALF):
                for b in range(ABUFS):
                    a = acps[h][b]
                    nc.gpsimd.memset(a[:, CSZ:CSZ + 2 * PITCH], 0.0)
                    nc.gpsimd.memset(a[:, 2 * CSZ - 2 * PITCH:2 * CSZ], 0.0)

            # bulk pair loads: all on the gpsimd ring, prologue-issued in
            # consumption order — the ring streams them continuously
            # while sync (shifts) and scalar (Signs) stay clear
            load_xr(0)
            load_xn(1)
            load_xr(1)
            load_xn(2)
            load_xr(2)
            load_xn(3)
            load_xr(3)

            def window(acp, offset, step, ncols):
                """Flat window AP [128, (2,)? ncols] of the copies tile;
                step is the DoubleRow pair stride (None for single)."""
                base = acp[:, offset:offset + 1]
                ap = [list(acp[:].ap[0])]
                if step is not None:
                    ap.append([step, 2])
                ap.append([1, ncols])
                return bass.AP(base.tensor, base.offset, ap)

            for n in range(n_samples):
                pi, sl = divmod(n, 2)
                xn = xn_t[pi]

                ps = psump.tile([128, NHALF, NCHUNK, NMM], fp32,
                                name="ps", tag="ps")
                for h in range(NHALF):
                    a = acps[h][n % ABUFS]
                    a3 = a[:].rearrange("p (k r x) -> p k r x", k=3,
                                        x=PITCH)
                    # a = Sign(x*scale + bias), fp8, into copy1 interior
                    nc.scalar.activation(
                        a[:, ACT_OFF:ACT_OFF + S],
                        xn[:, sl, h, :],
                        mybir.ActivationFunctionType.Sign,
                        bias=bi_tile[:, h:h + 1],
                        scale=sc_tile[:, h:h + 1],
                    )
                    # flat shifted copies (SBUF->SBUF DMA: one contiguous
                    # 1150B run per partition; engine copies are ~3.6
                    # ns/elem, far too slow) + edge re-zeroing
                    nc.sync.dma_start(a[:, 2:CSZ],
                                      a[:, CSZ:2 * CSZ - 2])
                    nc.sync.dma_start(a[:, 2 * CSZ:3 * CSZ - 2],
                                      a[:, CSZ + 2:2 * CSZ])
                    nc.gpsimd.memset(a3[:, 2, :, W - 2:W], 0.0)
                    nc.vector.memset(a3[:, 0, :, 0:2], 0.0)

                # conv: 4 DoubleRow pairs + 1 single, pass-major so
                # consecutive MMs share lhsT; N=512 per chunk
                for h in range(NHALF):
                    a = acps[h][n % ABUFS]
                    for p in range(4):
                        dy0, dx0 = SLOT_MAP[2 * p]
                        dy1, dx1 = SLOT_MAP[2 * p + 1]
                        off0 = dx0 * CSZ + 2 * dy0 * PITCH
                        step = (dx1 * CSZ + 2 * dy1 * PITCH) - off0
                        for c in range(NCHUNK):
                            nc.tensor.matmul(
                                ps[:, h, c, :],
                                w_tile[:, h * 9 + 2 * p:h * 9 + 2 * p + 2,
                                       :],
                                window(a, off0 + c * NMM, step, NMM),
                                start=(p == 0), stop=False,
                                perf_mode=mybir.MatmulPerfMode.DoubleRow,
                            )
                    dy0, dx0 = SLOT_MAP[8]
                    off0 = dx0 * CSZ + 2 * dy0 * PITCH
                    for c in range(NCHUNK):
                        nc.tensor.matmul(
                            ps[:, h, c, :],
                            w_tile[:, h * 9 + 8, :],
                            window(a, off0 + c * NMM, None, NMM),
                            start=False, stop=(c == NCHUNK - 1),
                        )

                # fused eviction + shuffle + residual (psum is exact and
                # contiguous: [h, c, 512] matches [h, s] linear order)
                if sl == 0:
                    fin = finp.tile([128, 2, NHALF, S], bf16, name="fin",
                                    tag="fin")
                else:
                    fin = fin_prev
                fin_prev = fin
                xr = xr_t[pi]
                for h in range(NHALF):
                    nc.vector.tensor_add(
                        fin[:, sl, h, :],
                        ps[:, h, :, :].rearrange("p c b -> p (c b)"),
                        xr[:, sl, h, :])
                    # per-half store overlaps the other half's add (tail)
                    nc.scalar.dma_start(out[n, :, h, :], fin[:, sl, h, :])
                if sl == 1:
                    xn_t.pop(pi)
                    xr_t.pop(pi)

    nc.compile()
    return nc


def _host_prep(x, weight, gamma, beta, running_mean, running_var):
    """Precompute BN affine + block-diagonal signed weights."""
    inv = (gamma / np.sqrt(running_var + EPS)).astype(np.float32)
    bias = (beta - running_mean * inv).astype(np.float32)
    wsign = np.sign(weight).astype(np.float32)   # [256, 4, 3, 3]

    lhsT = np.zeros((NHALF, K * K, 128, 128), np.float32)
    # Column m of lhsT (-> PSUM partition m) holds cout co = 4*(m%32)+m//32
    # within the half, so PSUM partition order is m = 32j + g for conv
    # cout 4g + j; the shuffled final channel is then 64j + 32h + g.
    m = np.arange(128)
    co = CPG * (m % 32) + m // 32
    gl = co // CPG
    for h in range(NHALF):
        for s, (dy, dx) in enumerate(SLOT_MAP):
            for kk in range(CPG):
                lhsT[h, s, CPG * gl + kk, m] = wsign[128 * h + co, kk,
                                                     dy, dx]
    lhsT = np.ascontiguousarray(
        lhsT.astype(ml_dtypes.float8_e4m3)
        .transpose(2, 0, 1, 3)
        .reshape(128, NHALF * K * K, 128))
    sc = np.ascontiguousarray(inv.reshape(NHALF, 128))
    bi = np.ascontiguousarray(bias.reshape(NHALF, 128))
    return lhsT, sc, bi


def _get_compiled():
    global _COMPILED
    if _COMPILED is None:
        _COMPILED = build(NS)
    return _COMPILED


# channel permutation: psum partition m, slot h <-> channel 64j + 32h + g
_PM = np.arange(128)
_PERM = (64 * (_PM[:, None] // 32) + 32 * np.arange(2)[None, :]
         + _PM[:, None] % 32)          # [128, 2] -> channel index


def make_in_maps(x, weight, gamma, beta, running_mean, running_var):
    lhsT, sc, bi = _host_prep(x, weight, gamma, beta, running_mean,
                              running_var)
    xs = x.astype(np.float32).reshape(N_CORES, NS, C, S)
    # sign path: natural, partition-major [ns, 128, 2, S]
    xns = np.ascontiguousarray(
        xs.astype(np.float16).reshape(N_CORES, NS, NHALF, 128, S)
        .transpose(0, 1, 3, 2, 4))
    # residual path: psum-order permuted [ns, 128, 2, S]
    xrs = np.ascontiguousarray(
        xs.astype(ml_dtypes.bfloat16)[:, :, _PERM, :])
    return [
        {"xin": xns[i], "xres": xrs[i], "wT": lhsT, "bnsc": sc, "bnbi": bi}
        for i in range(N_CORES)
    ]


def kernel(x, weight, gamma, beta, running_mean, running_var):
    from concourse.bass_utils import run_bass_kernel_spmd

    nc = _get_compiled()
    in_maps = make_in_maps(np.asarray(x), np.asarray(weight),
                           np.asarray(gamma), np.asarray(beta),
                           np.asarray(running_mean), np.asarray(running_var))
    res = run_bass_kernel_spmd(nc, in_maps, list(range(N_CORES)))
    outs = []
    for i in range(N_CORES):
        od = res.results[i]["out"].astype(np.float32)   # [NS, 128, 2, S]
        on = np.empty((NS, C, S), np.float32)
        on[:, _PERM.reshape(-1), :] = od.reshape(NS, 256, S)
        outs.append(on.reshape(NS, C, H, W))
    return np.concatenate(outs, axis=0)


# revision 37
# speedup vs baseline: 1.1673x; 1.1586x over previous
"""BinaryDilGroupConv Trainium2 kernel (v4).

Computes, for x[N=64, C=256, 32, 32]:
    h = BN(x)  (inference affine)
    a = sign(h); w = sign(weight)
    y = grouped dilated conv(a, w; groups=64, k=3, dil=2, pad=2)
    out = channel_shuffle(y, g=64) + x

Sharding: data-parallel over batch N across 8 NeuronCores (8 samples/core).
Params replicated. No collectives.

v4 design:
  - Activations live at row pitch 32 (no x padding), as THREE copies, one
    per dx tap column-shift (built from the Sign output with two flat
    1150-byte shifted copies + small edge memsets). Flat matmul windows
    are then EXACT: 512 columns per 16-row chunk, zero junk, and every
    window offset is a multiple of 32 so any two taps can form an fp8
    DoubleRow pair (step%16==0). 9 taps run as 4 DoubleRow pairs + 1
    single = 5 passes of N=512 (vs 6 passes of N=320+junk in v1-v3).
  - PSUM eviction + channel shuffle + residual add stay fused in one DVE
    tensor_add per sample: psum holds couts at m = 32j+g (shuffled final
    channel 64j+32h+g); a second copy of x is uploaded host-permuted to
    that order, so the add lines up partition-for-partition, and the
    host un-permutes the bf16 output after download.
  - All per-pair DRAM I/O is ONE dma trigger (host pre-permutes layouts;
    queue-side trigger cost ~0.7us dominates descriptor shape).
  - x for the sign path is fp16 (sign flips vs fp32: ~1e-5 of elements,
    rel err ~5e-3 << 2e-2); residual copy + output are bf16.
"""

import numpy as np
import ml_dtypes

C = 256
G = 64            # groups
CPG = 4           # channels per group
K = 3
DIL = 2
PAD = 2
EPS = 1e-5
H = W = 32
S = H * W         # 1024 spatial positions
PITCH = 32        # activation row pitch (exact, no x padding)
CPH = 36          # copy rows: 2 pad + 32 + 2 pad
CSZ = CPH * PITCH  # 1152 bytes per dx-copy
ACT_OFF = CSZ + 2 * PITCH   # interior of copy 1 (the unshifted one)
N_FULL = 64
N_CORES = 8
NS = N_FULL // N_CORES   # samples per core
NHALF = 2                # channel halves of 128
NCHUNK = 2               # psum chunks per half (16 output rows each)
CROWS = H // NCHUNK      # 16 rows per chunk
NMM = CROWS * PITCH      # 512 columns per matmul (one psum bank)
ABUFS = 3                # activation-copy round-robin depth
NPAIR = NS // 2

# tap slot order: 4 DoubleRow pairs + 1 single; (dy, dx) per slot
SLOT_MAP = [(0, 0), (1, 0), (0, 1), (1, 1), (0, 2), (1, 2),
            (2, 0), (2, 1), (2, 2)]

_COMPILED = None


def build(n_samples=NS):
    """Build + compile the per-core Bass program."""
    import concourse.bass as bass
    import concourse.bacc as bacc
    import concourse.tile as tile
    import concourse.mybir as mybir

    fp32 = mybir.dt.float32
    bf16 = mybir.dt.bfloat16
    fp16 = mybir.dt.float16
    fp8 = mybir.dt.float8e4

    nc = bacc.Bacc("TRN2", target_bir_lowering=False, debug=False,
                   num_devices=N_CORES)

    # sign path: natural channel order, partition-major [p=ch%128, h, s]
    xin = nc.dram_tensor("xin", [n_samples, 128, NHALF, S], fp16,
                         kind="ExternalInput").ap()
    # residual path: host-permuted to psum order (partition m = 32j+g,
    # slot h holds channel 64j+32h+g)
    xres = nc.dram_tensor("xres", [n_samples, 128, NHALF, S], bf16,
                          kind="ExternalInput").ap()
    # weight free index = h*9 + slot (see SLOT_MAP)
    wT = nc.dram_tensor("wT", [128, NHALF * K * K, 128], fp8,
                        kind="ExternalInput").ap()
    bnsc = nc.dram_tensor("bnsc", [NHALF, 128], fp32,
                          kind="ExternalInput").ap()
    bnbi = nc.dram_tensor("bnbi", [NHALF, 128], fp32,
                          kind="ExternalInput").ap()
    # output in psum order; host un-permutes
    out = nc.dram_tensor("out", [n_samples, 128, NHALF, S], bf16,
                         kind="ExternalOutput").ap()

    with tile.TileContext(nc) as tc:
        with (
            tc.tile_pool(name="const", bufs=1) as constp,
            tc.tile_pool(name="xnp", bufs=NPAIR) as xnp,
            tc.tile_pool(name="xrp", bufs=NPAIR) as xrp,
            tc.tile_pool(name="finp", bufs=2) as finp,
            tc.tile_pool(name="psum", bufs=2, space="PSUM") as psump,
        ):
            xn_t = {}
            xr_t = {}

            def load_xn(pi, split=False):
                xn_t[pi] = xnp.tile([128, 2, NHALF, S], fp16, name="xn",
                                    tag="xn")
                src = xin[2 * pi:2 * pi + 2].rearrange("n p h s -> p n h s")
                if split:
                    # sample 0 h0 races ahead, split across two rings;
                    # the rest of the pair follows
                    nc.sync.dma_start(xn_t[pi][:, 0, 0, 0:S // 2],
                                      src[:, 0, 0, 0:S // 2])
                    nc.gpsimd.dma_start(xn_t[pi][:, 0, 0, S // 2:S],
                                        src[:, 0, 0, S // 2:S])
                    nc.sync.dma_start(xn_t[pi][:, 0, 1, :],
                                      src[:, 0, 1, :])
                    nc.scalar.dma_start(xn_t[pi][:, 1, :, :],
                                        src[:, 1, :, :])
                else:
                    eng = nc.sync if pi % 2 else nc.scalar
                    eng.dma_start(xn_t[pi][:], src)

            def load_xr(pi):
                xr_t[pi] = xrp.tile([128, 2, NHALF, S], bf16, name="xr",
                                    tag="xr")
                nc.sync.dma_start(
                    xr_t[pi][:],
                    xres[2 * pi:2 * pi + 2].rearrange("n p h s -> p n h s"))

            sc_tile = constp.tile([128, NHALF], fp32)
            nc.scalar.dma_start(sc_tile[:], bnsc.rearrange("h p -> p h"))
            bi_tile = constp.tile([128, NHALF], fp32)
            nc.scalar.dma_start(bi_tile[:], bnbi.rearrange("h p -> p h"))
            load_xn(0, split=True)
            w_tile = constp.tile([128, NHALF * K * K, 128], fp8)
            nc.scalar.dma_start(w_tile[:], wT)

            # warmup: ACT table load + PE busy until the first real MM so
            # HAM is un-throttled by then
            warm_sb = constp.tile([128, 512], fp8)
            nc.gpsimd.memset(warm_sb[:], 0.0)
            warm_w = constp.tile([128, 128], fp8)
            nc.gpsimd.memset(warm_w[:], 0.0)
            warm_act = constp.tile([128, 16], fp8)
            nc.scalar.activation(warm_act[:], warm_sb[:, 0:16],
                                 mybir.ActivationFunctionType.Sign)
            warm_ps = psump.tile([128, NHALF, NCHUNK, NMM], fp32,
                                 name="ps", tag="ps")
            for i in range(8):
                nc.tensor.matmul(warm_ps[:, i % 2, (i // 2) % 2, :],
                                 warm_w[:], warm_sb[:],
                                 start=True, stop=True)
            for i in range(4):
                nc.tensor.matmul(warm_ps[:, i % 2, (i // 2) % 2, :],
                                 w_tile[:, 0, :], warm_sb[:],
                                 start=True, stop=True)

            # ---- activation copies: [copy0 | copy1 | copy2], pitch 32.
            # copy k serves dx=k taps: copy_k[r, x] = a_img[r-2, x+2k-2].
            # copy1 rows 0,1,34,35 are static zeros; the flat shifted
            # copies propagate them; edge cols are re-zeroed per sample.
            acps = [[constp.tile([128, 3 * CSZ], fp8, name=f"ac{h}_{b}")
                     for b in range(ABUFS)] for h in range(NHALF)]
            for h in range(NHALF):
                for b in range(ABUFS):
                    a = acps[h][b]
                    nc.gpsimd.memset(a[:, CSZ:CSZ + 2 * PITCH], 0.0)
                    nc.gpsimd.memset(a[:, 2 * CSZ - 2 * PITCH:2 * CSZ], 0.0)

            # bulk pair loads: prologue-issued in consumption order on
            # the sync/scalar rings (gpsimd stays clear for the
            # latency-critical shift DMAs)
            load_xr(0)
            load_xn(1)
            load_xr(1)

            def window(acp, offset, step, ncols):
                """Flat window AP [128, (2,)? ncols] of the copies tile;
                step is the DoubleRow pair stride (None for single)."""
                base = acp[:, offset:offset + 1]
                ap = [list(acp[:].ap[0])]
                if step is not None:
                    ap.append([step, 2])
                ap.append([1, ncols])
                return bass.AP(base.tensor, base.offset, ap)

            for n in range(n_samples):
                pi, sl = divmod(n, 2)
                xn = xn_t[pi]

                ps = psump.tile([128, NHALF, NCHUNK, NMM], fp32,
                                name="ps", tag="ps")
                for h in range(NHALF):
                    a = acps[h][n % ABUFS]
                    a3 = a[:].rearrange("p (k r x) -> p k r x", k=3,
                                        x=PITCH)
                    # a = Sign(x*scale + bias), fp8, into copy1 interior
                    nc.scalar.activation(
                        a[:, ACT_OFF:ACT_OFF + S],
                        xn[:, sl, h, :],
                        mybir.ActivationFunctionType.Sign,
                        bias=bi_tile[:, h:h + 1],
                        scale=sc_tile[:, h:h + 1],
                    )
                    # flat shifted copies (SBUF->SBUF DMA: one contiguous
                    # 1150B run per partition; engine copies are ~3.6
                    # ns/elem, far too slow) + edge re-zeroing
                    nc.gpsimd.dma_start(a[:, 2:CSZ],
                                        a[:, CSZ:2 * CSZ - 2])
                    nc.gpsimd.dma_start(a[:, 2 * CSZ:3 * CSZ - 2],
                                        a[:, CSZ + 2:2 * CSZ])
                    nc.gpsimd.memset(a3[:, 2, :, W - 2:W], 0.0)
                    nc.vector.memset(a3[:, 0, :, 0:2], 0.0)

                # stagger the remaining bulk loads (~2 pairs of lead)
                if n == 1:
                    load_xn(2)
                    load_xr(2)
                elif n == 3:
                    load_xn(3)
                    load_xr(3)

                # conv: 4 DoubleRow pairs + 1 single, pass-major so
                # consecutive MMs share lhsT; N=512 per chunk
                for h in range(NHALF):
                    a = acps[h][n % ABUFS]
                    for p in range(4):
                        dy0, dx0 = SLOT_MAP[2 * p]
                        dy1, dx1 = SLOT_MAP[2 * p + 1]
                        off0 = dx0 * CSZ + 2 * dy0 * PITCH
                        step = (dx1 * CSZ + 2 * dy1 * PITCH) - off0
                        for c in range(NCHUNK):
                            nc.tensor.matmul(
                                ps[:, h, c, :],
                                w_tile[:, h * 9 + 2 * p:h * 9 + 2 * p + 2,
                                       :],
                                window(a, off0 + c * NMM, step, NMM),
                                start=(p == 0), stop=False,
                                perf_mode=mybir.MatmulPerfMode.DoubleRow,
                            )
                    dy0, dx0 = SLOT_MAP[8]
                    off0 = dx0 * CSZ + 2 * dy0 * PITCH
                    for c in range(NCHUNK):
                        nc.tensor.matmul(
                            ps[:, h, c, :],
                            w_tile[:, h * 9 + 8, :],
                            window(a, off0 + c * NMM, None, NMM),
                            start=False, stop=(c == NCHUNK - 1),
                        )

                # fused eviction + shuffle + residual (psum is exact and
                # contiguous: [h, c, 512] matches [h, s] linear order)
                if sl == 0:
                    fin = finp.tile([128, 2, NHALF, S], bf16, name="fin",
                                    tag="fin")
                else:
                    fin = fin_prev
                fin_prev = fin
                xr = xr_t[pi]
                for h in range(NHALF):
                    nc.vector.tensor_add(
                        fin[:, sl, h, :],
                        ps[:, h, :, :].rearrange("p c b -> p (c b)"),
                        xr[:, sl, h, :])
                # per-sample store keeps the final transfer small (tail)
                nc.sync.dma_start(out[n], fin[:, sl, :, :])
                if sl == 1:
                    xn_t.pop(pi)
                    xr_t.pop(pi)

    nc.compile()
    return nc


def _host_prep(x, weight, gamma, beta, running_mean, running_var):
    """Precompute BN affine + block-diagonal signed weights."""
    inv = (gamma / np.sqrt(running_var + EPS)).astype(np.float32)
    bias = (beta - running_mean * inv).astype(np.float32)
    wsign = np.sign(weight).astype(np.float32)   # [256, 4, 3, 3]

    lhsT = np.zeros((NHALF, K * K, 128, 128), np.float32)
    # Column m of lhsT (-> PSUM partition m) holds cout co = 4*(m%32)+m//32
    # within the half, so PSUM partition order is m = 32j + g for conv
    # cout 4g + j; the shuffled final channel is then 64j + 32h + g.
    m = np.arange(128)
    co = CPG * (m % 32) + m // 32
    gl = co // CPG
    for h in range(NHALF):
        for s, (dy, dx) in enumerate(SLOT_MAP):
            for kk in range(CPG):
                lhsT[h, s, CPG * gl + kk, m] = wsign[128 * h + co, kk,
                                                     dy, dx]
    lhsT = np.ascontiguousarray(
        lhsT.astype(ml_dtypes.float8_e4m3)
        .transpose(2, 0, 1, 3)
        .reshape(128, NHALF * K * K, 128))
    sc = np.ascontiguousarray(inv.reshape(NHALF, 128))
    bi = np.ascontiguousarray(bias.reshape(NHALF, 128))
    return lhsT, sc, bi


def _get_compiled():
    global _COMPILED
    if _COMPILED is None:
        _COMPILED = build(NS)
    return _COMPILED


# channel permutation: psum partition m, slot h <-> channel 64j + 32h + g
_PM = np.arange(128)
_PERM = (64 * (_PM[:, None] // 32) + 32 * np.arange(2)[None, :]
         + _PM[:, None] % 32)          # [128, 2] -> channel index


def make_in_maps(x, weight, gamma, beta, running_mean, running_var):
    lhsT, sc, bi = _host_prep(x, weight, gamma, beta, running_mean,
                              running_var)
    xs = x.astype(np.float32).reshape(N_CORES, NS, C, S)
    # sign path: natural, partition-major [ns, 128, 2, S]
    xns = np.ascontiguousarray(
        xs.astype(np.float16).reshape(N_CORES, NS, NHALF, 128, S)
        .transpose(0, 1, 3, 2, 4))
    # residual path: psum-order permuted [ns, 128, 2, S]
    xrs = np.ascontiguousarray(
        xs.astype(ml_dtypes.bfloat16)[:, :, _PERM, :])
    return [
        {"xin": xns[i], "xres": xrs[i], "wT": lhsT, "bnsc": sc, "bnbi": bi}
        for i in range(N_CORES)
    ]


def kernel(x, weight, gamma, beta, running_mean, running_var):
    from concourse.bass_utils import run_bass_kernel_spmd

    nc = _get_compiled()
    in_maps = make_in_maps(np.asarray(x), np.asarray(weight),
                           np.asarray(gamma), np.asarray(beta),
                           np.asarray(running_mean), np.asarray(running_var))
    res = run_bass_kernel_spmd(nc, in_maps, list(range(N_CORES)))
    outs = []
    for i in range(N_CORES):
        od = res.results[i]["out"].astype(np.float32)   # [NS, 128, 2, S]
        on = np.empty((NS, C, S), np.float32)
        on[:, _PERM.reshape(-1), :] = od.reshape(NS, 256, S)
        outs.append(on.reshape(NS, C, H, W))
    return np.concatenate(outs, axis=0)
